# revision 1
# baseline (speedup 1.0000x reference)
"""Multi-head attention (B=2, S=2048, D=768, H=12) on 8 Trainium2 cores.

Sharding: core c handles batch b=c//4 and heads 3*(c%4)..3*(c%4)+3.
QKV weights column-sharded, Wo row-sharded (Megatron); host sums the 4
partial outputs per batch and adds bo.

Per-core kernel (all fp32r matmuls; scores computed transposed so softmax
needs no on-chip transposes; softmax denominator via a ones column in V):
  KT/QT[192,2048] d-major, V[2048,192] k-major (+ones cols)
  S^T[k,q] = K @ Q^T per head (K=64 matmuls row-packed in pairs via
  PE row groups so two run concurrently on the array)
  P = exp(S^T/8)  (no max subtraction; scores are O(10) so exp is safe)
  out^T[65,512] += [V_h|1]^T @ P^T accumulating over k; row 64 = denom
  normalize by 1/den, y_part = out^T.T @ Wo_rows
"""

import sys

sys.path.insert(0, "/opt/trn_rl_repo")

from contextlib import ExitStack

import numpy as np

import concourse.bacc as bacc
import concourse.bass as bass
import concourse.tile as tile
from concourse import mybir
from concourse.bass_utils import run_bass_kernel_spmd

F32 = mybir.dt.float32
F32R = mybir.dt.float32r

S = 2048  # sequence length
D = 768  # model dim
HP = 3  # heads per core
DK = 64  # head dim
DO = HP * DK  # 192 out-cols per core
KT = D // 128  # 6 contraction tiles for projections
NB = S // 512  # 4 sequence blocks of 512
NKT = S // 128  # 16 kpos tiles
G = NKT // 2  # 8 groups of 2 kpos tiles
VW = HP * 65 + 1  # 196: [1|V0|1|V1|1|V2|1] ones interleaved


def emit_kernel(nc, tc, t, reps=1):
    ctx = ExitStack()
    sb = ctx.enter_context(tc.tile_pool(name="sb", bufs=1))
    xp = ctx.enter_context(tc.tile_pool(name="xp", bufs=1))
    pt_pool = ctx.enter_context(tc.tile_pool(name="ptp", bufs=2))
    work = ctx.enter_context(tc.tile_pool(name="work", bufs=2))
    ps = ctx.enter_context(tc.tile_pool(name="ps", bufs=1, space=bass.MemorySpace.PSUM))

    # ---- persistent SBUF tensors ----
    wq_sb = sb.tile([128, KT, DO], F32R)
    wk_sb = sb.tile([128, KT, DO], F32R)
    wv_sb = sb.tile([128, KT, DO], F32R)
    wo1_sb = sb.tile([128, D], F32R)  # Wo rows 0:128
    wo2_sb = sb.tile([64, D], F32R)  # Wo rows 128:192
    bq_sb = sb.tile([128, 2], F32)
    bk_sb = sb.tile([128, 2], F32)
    bv_bc = sb.tile([128, DO], F32)  # bv broadcast to 128 partitions
    qt_a = sb.tile([128, S], F32R)  # Q^T rows 0:128 (heads 0,1)
    qt_b = sb.tile([128, S], F32R)  # Q^T rows 128:192 (head 2; dup at 64:128)
    kt_a = sb.tile([128, S], F32R)
    kt_b = sb.tile([128, S], F32R)
    v_sb = sb.tile([128, NKT, VW], F32R)  # k-major V with ones cols
    out_a = sb.tile([128, S], F32R)  # attention out^T rows 0:128
    out_b = sb.tile([64, S], F32R)  # rows 128:192

    # ---- weight / bias loads (once) ----
    nc.sync.dma_start(wq_sb[:], t["wq"].ap().rearrange("(k p) o -> p k o", p=128))
    nc.sync.dma_start(wk_sb[:], t["wk"].ap().rearrange("(k p) o -> p k o", p=128))
    nc.sync.dma_start(wv_sb[:], t["wv"].ap().rearrange("(k p) o -> p k o", p=128))
    nc.sync.dma_start(wo1_sb[:], t["wo"].ap()[0:128, :])
    nc.sync.dma_start(wo2_sb[:], t["wo"].ap()[128:DO, :])
    nc.sync.dma_start(bq_sb[:, 0:1], t["bq"].ap()[0:128, :])
    nc.sync.dma_start(bq_sb[0:64, 1:2], t["bq"].ap()[128:DO, :])
    nc.sync.dma_start(bk_sb[:, 0:1], t["bk"].ap()[0:128, :])
    nc.sync.dma_start(bk_sb[0:64, 1:2], t["bk"].ap()[128:DO, :])
    nc.sync.dma_start(bv_bc[:], t["bv"].ap().unsqueeze(1).to_broadcast([1, 128, DO]))
    for oc in (0, 65, 130, 195):
        nc.vector.memset(v_sb[:, :, oc : oc + 1].bitcast(F32), 1.0)

    def load_x(xt_dram):
        xts = []
        for k in range(KT):
            xt = xp.tile([128, S], F32R, name=f"xt{k}", tag=f"xt{k}")
            nc.gpsimd.dma_start(xt[:], xt_dram.ap()[k * 128 : k * 128 + 128, :])
            xts.append(xt)
        return xts

    def proj_qk(xts, w_sb, b_sb, dst_a, dst_b):
        for nb in range(NB):
            nb0 = nb * 512
            pq1 = ps.tile([128, 2, 512], F32, tag="A", bufs=3, name="pq1")
            for k in range(KT):
                nc.tensor.matmul(
                    pq1[:, 0, :], w_sb[:, k, 0:128], xts[k][:, nb0 : nb0 + 512],
                    start=(k == 0), stop=(k == KT - 1),
                )
            nc.vector.tensor_scalar_add(
                dst_a[:, nb0 : nb0 + 512], pq1[:, 0, :], b_sb[:, 0:1]
            )
            pq2 = ps.tile([64, 512], F32, tag="B", bufs=2, name="pq2")
            for k in range(KT):
                nc.tensor.matmul(
                    pq2[:], w_sb[:, k, 128:DO], xts[k][:, nb0 : nb0 + 512],
                    start=(k == 0), stop=(k == KT - 1),
                )
            nc.vector.tensor_scalar_add(
                dst_b[0:64, nb0 : nb0 + 512], pq2[:], b_sb[0:64, 1:2]
            )
            # duplicate head-2 rows into partitions 64:128 for row-packing
            nc.sync.dma_start(
                dst_b[64:128, nb0 : nb0 + 512], dst_b[0:64, nb0 : nb0 + 512]
            )

    def proj_v(xts):
        for st in range(16):
            pv = ps.tile([128, DO], F32, tag="B", bufs=2, name="pv")
            for k in range(KT):
                nc.tensor.matmul(
                    pv[:], xts[k][:, st * 128 : st * 128 + 128], wv_sb[:, k, :],
                    start=(k == 0), stop=(k == KT - 1),
                )
            dst = v_sb[:, st, 1:196].rearrange("p (h c) -> p h c", h=HP)[:, :, 0:64]
            nc.vector.tensor_add(dst, pv[:].rearrange("p (h c) -> p h c", h=HP),
                                 bv_bc[:].rearrange("p (h c) -> p h c", h=HP))

    def vslice(kt_i, h):
        return v_sb[:, kt_i, 1 + h * 65 : 1 + h * 65 + 65]

    def normalize(po, h, q0):
        """row 64 of po = denominator; write normalized out^T rows."""
        dtmp = work.tile([65, 512], F32, name="dtmp", tag="dtmp")
        nc.vector.reciprocal(dtmp[64:65, :], po[64:65, :])
        dbc = work.tile([64, 512], F32, name="dbc", tag="dbc")
        nc.sync.dma_start(dbc[:], dtmp[64:65, :].unsqueeze(1).to_broadcast([1, 64, 512]))
        if h == 0:
            nc.vector.tensor_mul(out_a[0:64, q0 : q0 + 512], po[0:64, :], dbc[:])
        elif h == 2:
            nc.vector.tensor_mul(out_b[0:64, q0 : q0 + 512], po[0:64, :], dbc[:])
        else:
            nsb = work.tile([64, 512], F32R, name="nsb", tag="nsb")
            nc.vector.tensor_mul(nsb[:], po[0:64, :], dbc[:])
            nc.sync.dma_start(out_a[64:128, q0 : q0 + 512], nsb[:])

    def attention():
        for qb in range(NB):
            q0 = qb * 512
            # heads 0 and 1 together: K=64 score matmuls at PE row groups
            # 0-1 / 2-3 run concurrently when emitted back-to-back.
            po0 = ps.tile([65, 512], F32, tag="B", bufs=2, name="po0")
            po1 = ps.tile([65, 512], F32, tag="B", bufs=2, name="po1")
            for g in range(G):
                pss0 = ps.tile([128, 2, 512], F32, tag="A", bufs=3, name="pss0")
                pss1 = ps.tile([128, 2, 512], F32, tag="A", bufs=3, name="pss1")
                for kl in range(2):
                    kk = (g * 2 + kl) * 128
                    nc.tensor.matmul(
                        pss0[:, kl, :], kt_a[0:64, kk : kk + 128],
                        qt_a[0:64, q0 : q0 + 512], start=True, stop=True,
                    )
                    nc.tensor.matmul(
                        pss1[:, kl, :], kt_a[64:128, kk : kk + 128],
                        qt_a[64:128, q0 : q0 + 512], start=True, stop=True,
                    )
                pt0 = pt_pool.tile([128, 2, 512], F32R, name="pt0", tag="pt0")
                nc.scalar.activation(
                    pt0[:], pss0[:], mybir.ActivationFunctionType.Exp, scale=0.125
                )
                pt1 = pt_pool.tile([128, 2, 512], F32R, name="pt1", tag="pt1")
                nc.scalar.activation(
                    pt1[:], pss1[:], mybir.ActivationFunctionType.Exp, scale=0.125
                )
                for kl in range(2):
                    kt_i = g * 2 + kl
                    nc.tensor.matmul(
                        po0[:], vslice(kt_i, 0), pt0[:, kl, :],
                        start=(kt_i == 0), stop=(kt_i == NKT - 1),
                        skip_group_check=True,
                    )
                    nc.tensor.matmul(
                        po1[:], vslice(kt_i, 1), pt1[:, kl, :],
                        start=(kt_i == 0), stop=(kt_i == NKT - 1),
                        skip_group_check=True,
                    )
            normalize(po0, 0, q0)
            normalize(po1, 1, q0)
            # head 2: pair even/odd kt via the duplicated rows 64:128
            po2 = ps.tile([65, 512], F32, tag="B", bufs=2, name="po2")
            for g in range(G):
                pss2 = ps.tile([128, 2, 512], F32, tag="A", bufs=3, name="pss2")
                kk = g * 256
                nc.tensor.matmul(
                    pss2[:, 0, :], kt_b[0:64, kk : kk + 128],
                    qt_b[0:64, q0 : q0 + 512], start=True, stop=True,
                )
                nc.tensor.matmul(
                    pss2[:, 1, :], kt_b[64:128, kk + 128 : kk + 256],
                    qt_b[64:128, q0 : q0 + 512], start=True, stop=True,
                )
                pt2 = pt_pool.tile([128, 2, 512], F32R, name="pt2", tag="pt2")
                nc.scalar.activation(
                    pt2[:], pss2[:], mybir.ActivationFunctionType.Exp, scale=0.125
                )
                for kl in range(2):
                    kt_i = g * 2 + kl
                    nc.tensor.matmul(
                        po2[:], vslice(kt_i, 2), pt2[:, kl, :],
                        start=(kt_i == 0), stop=(kt_i == NKT - 1),
                        skip_group_check=True,
                    )
            normalize(po2, 2, q0)
            # ---- output projection for this q-block ----
            for ss in range(4):
                s0 = q0 + ss * 128
                py = ps.tile([128, D], F32, tag="A", bufs=3, name="py")
                for nc0, nn in ((0, 512), (512, 256)):
                    nc.tensor.matmul(
                        py[:, nc0 : nc0 + nn], out_a[:, s0 : s0 + 128],
                        wo1_sb[:, nc0 : nc0 + nn], start=True, stop=False,
                    )
                    nc.tensor.matmul(
                        py[:, nc0 : nc0 + nn], out_b[:, s0 : s0 + 128],
                        wo2_sb[:, nc0 : nc0 + nn], start=False, stop=True,
                    )
                ysb = work.tile([128, D], F32, name="ysb", tag="ysb")
                nc.vector.tensor_copy(ysb[:], py[:])
                nc.sync.dma_start(t["y"].ap()[s0 : s0 + 128, :], ysb[:])

    for _ in range(reps):
        xts = load_x(t["xk"])
        proj_qk(xts, wk_sb, bk_sb, kt_a, kt_b)
        xts = load_x(t["xv"])
        proj_v(xts)
        xts = load_x(t["xq"])
        proj_qk(xts, wq_sb, bq_sb, qt_a, qt_b)
        attention()

    ctx.close()


_NC_CACHE = {}


def build_nc(reps=1):
    if reps in _NC_CACHE:
        return _NC_CACHE[reps]
    nc = bacc.Bacc("TRN2", target_bir_lowering=False, debug=False, num_devices=8)
    t = {}
    for name in ("xq", "xk", "xv"):
        t[name] = nc.dram_tensor(name, [D, S], F32R, kind="ExternalInput")
    for name in ("wq", "wk", "wv"):
        t[name] = nc.dram_tensor(name, [D, DO], F32R, kind="ExternalInput")
    t["wo"] = nc.dram_tensor("wo", [DO, D], F32R, kind="ExternalInput")
    for name in ("bq", "bk"):
        t[name] = nc.dram_tensor(name, [DO, 1], F32, kind="ExternalInput")
    t["bv"] = nc.dram_tensor("bv", [1, DO], F32, kind="ExternalInput")
    t["y"] = nc.dram_tensor("y", [S, D], F32, kind="ExternalOutput")

    with tile.TileContext(nc) as tc:
        emit_kernel(nc, tc, t, reps=reps)
    nc.compile()
    _NC_CACHE[reps] = nc
    return nc


def make_in_maps(q, k, v, Wq, bq, Wk, bk, Wv, bv, Wo, bo):
    in_maps = []
    for c in range(8):
        b = c // 4
        hs = (c % 4) * DO
        in_maps.append(
            {
                "xq": np.ascontiguousarray(q[b].T),
                "xk": np.ascontiguousarray(k[b].T),
                "xv": np.ascontiguousarray(v[b].T),
                "wq": np.ascontiguousarray(Wq[:, hs : hs + DO]),
                "wk": np.ascontiguousarray(Wk[:, hs : hs + DO]),
                "wv": np.ascontiguousarray(Wv[:, hs : hs + DO]),
                "wo": np.ascontiguousarray(Wo[hs : hs + DO, :]),
                "bq": np.ascontiguousarray(bq[hs : hs + DO, None]),
                "bk": np.ascontiguousarray(bk[hs : hs + DO, None]),
                "bv": np.ascontiguousarray(bv[None, hs : hs + DO]),
            }
        )
    return in_maps


def kernel(q, k, v, Wq, bq, Wk, bk, Wv, bv, Wo, bo, _reps=1):
    q = np.asarray(q, dtype=np.float32)
    k = np.asarray(k, dtype=np.float32)
    v = np.asarray(v, dtype=np.float32)
    nc = build_nc(reps=_reps)
    in_maps = make_in_maps(q, k, v, np.asarray(Wq), np.asarray(bq), np.asarray(Wk),
                           np.asarray(bk), np.asarray(Wv), np.asarray(bv),
                           np.asarray(Wo), np.asarray(bo))
    res = run_bass_kernel_spmd(nc, in_maps, list(range(8)))
    B = q.shape[0]
    y = np.zeros((B, S, D), dtype=np.float32)
    for c in range(8):
        y[c // 4] += res.results[c]["y"]
    y += np.asarray(bo, dtype=np.float32)[None, None, :]
    return y



# revision 5
# speedup vs baseline: 1.1444x; 1.1444x over previous
"""Multi-head attention (B=2, S=2048, D=768, H=12) on 8 Trainium2 cores.

Sharding: core c handles batch b=c//4 and heads 3*(c%4)..3*(c%4)+3.
QKV weights column-sharded, Wo row-sharded (Megatron); host sums the 4
partial outputs per batch and adds bo.

v2 schedule (per core):
  - x loaded in [128, seq-chunk] tiles over the sync HWDGE queue, ordered
    xk, xq(b0), xv(b01), xq(b1), xv(b23), xq(b2), xq(b3) so attention
    q-block 0 starts ~25us in while V still streams.
  - QK proj: psum [128,2,512] tile per nb; bank0 = head01 cols (128-wide
    matmuls), bank1 = head2 cols (64-wide); head2 rows duplicated to
    partitions 64:128 for PE row-tile pairing.
  - V proj: moving operand is Wv padded to 256 cols (fp32r needs moving
    >=256 for full PE rate); output evacuated to k-major bf16 V with
    interleaved ones columns (softmax denominator rides the PV matmul).
  - scores: all three heads' S^T matmuls are 64-row PE tiles; head0 at
    partitions 0:63 pairs with head1 at 64:127 in one psum tile (banks
    0/1); head2 pairs its even/odd kpos chunks via duplicated K/Q rows.
    exp (scale=1/8, no max subtraction) on ACT -> bf16 P^T tiles.
  - PV: full-height 128x128-mode matmuls, po[65,512] accumulators per
    head (row 64 = denominator), phased after the q-block's scores so the
    PE stays in one tile mode per phase.
  - normalize: reciprocal on DVE, broadcast via DMA, multiply on Pool;
    head1 shifted to partitions 64:127 by DMA; head2 lands at rows 0:64
    of a zero-padded [128,S] tensor so its 64-row output-proj matmul can
    run in 128x128 mode against a zero-padded Wo[128:192].
  - oproj: py [128,384] (one psum bank), evacuated on Pool, y written
    over the gpsimd SWDGE queue so rep r+1's x loads (sync queue) are
    not stuck behind rep r's y stores.
"""

import sys

sys.path.insert(0, "/opt/trn_rl_repo")

from contextlib import ExitStack

import numpy as np

import concourse.bacc as bacc
import concourse.bass as bass
import concourse.tile as tile
from concourse import mybir
from concourse.bass_utils import run_bass_kernel_spmd

F32 = mybir.dt.float32
F32R = mybir.dt.float32r
BF16 = mybir.dt.bfloat16

S = 2048  # sequence length
D = 768  # model dim
HP = 3  # heads per core
DK = 64  # head dim
DO = HP * DK  # 192 out-cols per core
KT = D // 128  # 6 contraction tiles for projections
NB = S // 512  # 4 sequence blocks of 512
NKT = S // 128  # 16 kpos tiles
G = NKT // 2  # 8 groups of 2 kpos tiles
VW = HP * 65 + 1  # 196: [1|V0|1|V1|1|V2|1] ones interleaved
EXP = mybir.ActivationFunctionType.Exp


def emit_kernel(nc, tc, t, reps=1):
    ctx = ExitStack()
    sb = ctx.enter_context(tc.tile_pool(name="sb", bufs=1))
    xa = ctx.enter_context(tc.tile_pool(name="xa", bufs=1))  # xk then xq
    xb = ctx.enter_context(tc.tile_pool(name="xb", bufs=1))  # xv
    ptp = ctx.enter_context(tc.tile_pool(name="ptp", bufs=1))
    work = ctx.enter_context(tc.tile_pool(name="work", bufs=2))
    ps = ctx.enter_context(tc.tile_pool(name="ps", bufs=1, space=bass.MemorySpace.PSUM))

    # ---- persistent SBUF tensors ----
    wq_sb = sb.tile([128, KT, DO], F32R)
    wk_sb = sb.tile([128, KT, DO], F32R)
    wv_sb = sb.tile([128, KT, 256], F32R)  # cols 192:256 zero (moving>=256)
    wo1_sb = sb.tile([128, D], F32R)  # Wo rows 0:128
    wo2b_sb = sb.tile([128, D], F32R)  # rows 0:64 = Wo[128:192], 64:128 zero
    bq_sb = sb.tile([128, 2], F32)
    bk_sb = sb.tile([128, 2], F32)
    bv_bc = sb.tile([128, DO], F32)  # bv broadcast to 128 partitions
    qt_a = sb.tile([128, S], F32R)  # Q^T rows 0:128 (heads 0,1)
    qt_b = sb.tile([128, S], F32R)  # head 2 at 0:64, dup at 64:128
    kt_a = sb.tile([128, S], F32R)
    kt_b = sb.tile([128, S], F32R)
    v_sb = sb.tile([128, NKT, VW], BF16)  # k-major V with ones cols
    out_a = sb.tile([128, S], F32R)  # normalized attn out^T, heads 0,1
    out_b2 = sb.tile([128, S], F32R)  # head 2 at rows 0:64, rows 64:128 zero

    # ---- one-time loads / inits ----
    nc.sync.dma_start(wq_sb[:], t["wq"].ap().rearrange("(k p) o -> p k o", p=128))
    nc.sync.dma_start(wk_sb[:], t["wk"].ap().rearrange("(k p) o -> p k o", p=128))
    nc.sync.dma_start(
        wv_sb[:, :, 0:DO], t["wv"].ap().rearrange("(k p) o -> p k o", p=128)
    )
    nc.vector.memset(wv_sb[:, :, DO:256].bitcast(F32), 0.0)
    nc.sync.dma_start(wo1_sb[:], t["wo"].ap()[0:128, :])
    nc.sync.dma_start(wo2b_sb[0:64, :], t["wo"].ap()[128:DO, :])
    nc.vector.memset(wo2b_sb[64:128, :].bitcast(F32), 0.0)
    nc.vector.memset(out_b2[64:128, :].bitcast(F32), 0.0)
    nc.sync.dma_start(bq_sb[:, 0:1], t["bq"].ap()[0:128, :])
    nc.sync.dma_start(bq_sb[0:64, 1:2], t["bq"].ap()[128:DO, :])
    nc.sync.dma_start(bk_sb[:, 0:1], t["bk"].ap()[0:128, :])
    nc.sync.dma_start(bk_sb[0:64, 1:2], t["bk"].ap()[128:DO, :])
    nc.sync.dma_start(bv_bc[:], t["bv"].ap().unsqueeze(1).to_broadcast([1, 128, DO]))
    for oc in (0, 65, 130, 195):
        nc.vector.memset(v_sb[:, :, oc : oc + 1], 1.0)

    def dma_x_nb(xdram, pool, pfx, nb):
        """Load x[:, nb*512:(nb+1)*512] as 6 [128,512] chunk tiles."""
        xts = []
        for k in range(KT):
            xt = pool.tile(
                [128, 512], F32R, name=f"{pfx}{k}", tag=f"{pfx}{k}", bufs=2
            )
            nc.sync.dma_start(
                xt[:], xdram.ap()[k * 128 : k * 128 + 128, nb * 512 : nb * 512 + 512]
            )
            xts.append(xt)
        return xts

    def proj_qk_nb(xts, w_sb, b_sb, dst_a, dst_b, nb):
        nb0 = nb * 512
        pq = ps.tile([128, 2, 512], F32, tag="ss", bufs=2, name="pq")
        for k in range(KT):
            nc.tensor.matmul(
                pq[:, 0, :], w_sb[:, k, 0:128], xts[k][:],
                start=(k == 0), stop=(k == KT - 1),
            )
        nc.vector.tensor_scalar_add(
            dst_a[:, nb0 : nb0 + 512], pq[:, 0, :], b_sb[:, 0:1]
        )
        for k in range(KT):
            nc.tensor.matmul(
                pq[0:64, 1, :], w_sb[:, k, 128:DO], xts[k][:],
                start=(k == 0), stop=(k == KT - 1),
            )
        nc.vector.tensor_scalar_add(
            dst_b[0:64, nb0 : nb0 + 512], pq[0:64, 1, :], b_sb[0:64, 1:2]
        )
        nc.scalar.dma_start(
            dst_b[64:128, nb0 : nb0 + 512], dst_b[0:64, nb0 : nb0 + 512]
        )

    def proj_v_nb(xts, nb):
        # tag "py" (not "ss") so the V pipeline never waits on score-tile
        # exp frees; bufs=1 serializes st on the previous evac, which hides
        # under the exp stream anyway.
        for sl in range(4):
            st = nb * 4 + sl
            pv = ps.tile([128, 256], F32, tag="py", bufs=1, name="pv")
            for k in range(KT):
                nc.tensor.matmul(
                    pv[:], xts[k][:, sl * 128 : sl * 128 + 128],
                    wv_sb[:, k, :], start=(k == 0), stop=(k == KT - 1),
                )
            dst = v_sb[:, st, 1:196].rearrange("p (h c) -> p h c", h=HP)[:, :, 0:64]
            nc.vector.tensor_add(
                dst,
                pv[:, 0:DO].rearrange("p (h c) -> p h c", h=HP),
                bv_bc[:].rearrange("p (h c) -> p h c", h=HP),
            )

    def vslice(kt_i, h):
        return v_sb[:, kt_i, 1 + h * 65 : 1 + h * 65 + 65]

    def scores_qb(qb):
        """All 3 heads' S^T for q-block qb; exp into bf16 P^T tiles.

        Returns pts[g] = (ptx, pty, ptz): ptx[:,j,:] = P^T of head j for
        kpos chunk 2g; pty for chunk 2g+1; ptz[:,0,:]/[:,1,:] = head 2
        chunks 2g / 2g+1."""
        q0 = qb * 512
        pts = []
        for g in range(G):
            ke, ko = 2 * g * 128, (2 * g + 1) * 128
            ssx = ps.tile([128, 2, 512], F32, tag="ss", bufs=2, name="ssx")
            nc.tensor.matmul(
                ssx[:, 0, :], kt_a[0:64, ke : ke + 128], qt_a[0:64, q0 : q0 + 512],
                start=True, stop=True,
            )
            nc.tensor.matmul(
                ssx[:, 1, :], kt_a[64:128, ke : ke + 128], qt_a[64:128, q0 : q0 + 512],
                start=True, stop=True,
            )
            ptx = ptp.tile([128, 2, 512], BF16, name="ptx", tag="pt", bufs=12)
            nc.scalar.activation(ptx[:], ssx[:], EXP, scale=0.125)
            ssy = ps.tile([128, 2, 512], F32, tag="ss", bufs=2, name="ssy")
            nc.tensor.matmul(
                ssy[:, 0, :], kt_a[0:64, ko : ko + 128], qt_a[0:64, q0 : q0 + 512],
                start=True, stop=True,
            )
            nc.tensor.matmul(
                ssy[:, 1, :], kt_a[64:128, ko : ko + 128], qt_a[64:128, q0 : q0 + 512],
                start=True, stop=True,
            )
            pty = ptp.tile([128, 2, 512], BF16, name="pty", tag="pt", bufs=12)
            nc.scalar.activation(pty[:], ssy[:], EXP, scale=0.125)
            ssz = ps.tile([128, 2, 512], F32, tag="ss", bufs=2, name="ssz")
            nc.tensor.matmul(
                ssz[:, 0, :], kt_b[0:64, ke : ke + 128], qt_b[0:64, q0 : q0 + 512],
                start=True, stop=True,
            )
            nc.tensor.matmul(
                ssz[:, 1, :], kt_b[64:128, ko : ko + 128], qt_b[64:128, q0 : q0 + 512],
                start=True, stop=True,
            )
            ptz = ptp.tile([128, 2, 512], BF16, name="ptz", tag="pt", bufs=12)
            nc.scalar.activation(ptz[:], ssz[:], EXP, scale=0.125)
            pts.append((ptx, pty, ptz))
        return pts

    def pv_qb(pts):
        """P^T @ [V|1] accumulated over kpos; returns po per head."""
        po0 = ps.tile([65, 512], F32, tag="po0", bufs=1, name="po0")
        po1 = ps.tile([65, 512], F32, tag="po1", bufs=1, name="po1")
        po2 = ps.tile([65, 512], F32, tag="po2", bufs=1, name="po2")
        for g in range(G):
            ptx, pty, ptz = pts[g]
            ke, ko = 2 * g, 2 * g + 1
            st, sp = g == 0, g == G - 1
            nc.tensor.matmul(po0[:], vslice(ke, 0), ptx[:, 0, :],
                             start=st, stop=False, skip_group_check=True)
            nc.tensor.matmul(po0[:], vslice(ko, 0), pty[:, 0, :],
                             start=False, stop=sp, skip_group_check=True)
            nc.tensor.matmul(po1[:], vslice(ke, 1), ptx[:, 1, :],
                             start=st, stop=False, skip_group_check=True)
            nc.tensor.matmul(po1[:], vslice(ko, 1), pty[:, 1, :],
                             start=False, stop=sp, skip_group_check=True)
            nc.tensor.matmul(po2[:], vslice(ke, 2), ptz[:, 0, :],
                             start=st, stop=False, skip_group_check=True)
            nc.tensor.matmul(po2[:], vslice(ko, 2), ptz[:, 1, :],
                             start=False, stop=sp, skip_group_check=True)
        return po0, po1, po2

    def normalize_qb(qb, po0, po1, po2):
        q0 = qb * 512
        for h, po in ((0, po0), (1, po1), (2, po2)):
            dtmp = work.tile([65, 512], F32, name="dtmp", tag="dtmp")
            nc.vector.reciprocal(dtmp[64:65, :], po[64:65, :])
            dbc = work.tile([64, 512], F32, name="dbc", tag="dbc")
            nc.scalar.dma_start(
                dbc[:], dtmp[64:65, :].unsqueeze(1).to_broadcast([1, 64, 512])
            )
            if h == 0:
                nc.vector.tensor_mul(out_a[0:64, q0 : q0 + 512], po[0:64, :], dbc[:])
            elif h == 2:
                nc.vector.tensor_mul(out_b2[0:64, q0 : q0 + 512], po[0:64, :], dbc[:])
            else:
                nsb = work.tile([64, 512], F32R, name="nsb", tag="nsb")
                nc.vector.tensor_mul(nsb[:], po[0:64, :], dbc[:])
                nc.scalar.dma_start(out_a[64:128, q0 : q0 + 512], nsb[:])

    def oproj_qb(qb):
        q0 = qb * 512
        for ss in range(4):
            s0 = q0 + ss * 128
            ysb = work.tile([128, D], F32, name="ysb", tag="ysb")
            for half in range(2):
                c0 = half * 384
                py = ps.tile([128, 384], F32, tag="py", bufs=1, name="py")
                nc.tensor.matmul(
                    py[:], out_a[:, s0 : s0 + 128], wo1_sb[:, c0 : c0 + 384],
                    start=True, stop=False,
                )
                nc.tensor.matmul(
                    py[:], out_b2[:, s0 : s0 + 128], wo2b_sb[:, c0 : c0 + 384],
                    start=False, stop=True,
                )
                nc.vector.tensor_copy(ysb[:, c0 : c0 + 384], py[:])
            nc.gpsimd.dma_start(t["y"].ap()[s0 : s0 + 128, :], ysb[:])

    for _ in range(reps):
        # K fully loaded+projected first, then q-block 0 of Q, then V
        # interleaved with the rest of Q (queue order = bandwidth order).
        for nb in range(NB):
            xts = dma_x_nb(t["xk"], xa, "a", nb)
            proj_qk_nb(xts, wk_sb, bk_sb, kt_a, kt_b, nb)
        xts = dma_x_nb(t["xq"], xa, "a", 0)
        proj_qk_nb(xts, wq_sb, bq_sb, qt_a, qt_b, 0)

        pts0 = scores_qb(0)

        for nb in range(2):
            xts = dma_x_nb(t["xv"], xb, "v", nb)
            proj_v_nb(xts, nb)
        xq1 = dma_x_nb(t["xq"], xa, "a", 1)
        for nb in range(2, NB):
            xts = dma_x_nb(t["xv"], xb, "v", nb)
            proj_v_nb(xts, nb)
        proj_qk_nb(xq1, wq_sb, bq_sb, qt_a, qt_b, 1)

        po = pv_qb(pts0)
        normalize_qb(0, *po)

        pts1 = scores_qb(1)
        oproj_qb(0)
        xts = dma_x_nb(t["xq"], xa, "a", 2)
        proj_qk_nb(xts, wq_sb, bq_sb, qt_a, qt_b, 2)
        po = pv_qb(pts1)
        normalize_qb(1, *po)

        pts2 = scores_qb(2)
        oproj_qb(1)
        xts = dma_x_nb(t["xq"], xa, "a", 3)
        proj_qk_nb(xts, wq_sb, bq_sb, qt_a, qt_b, 3)
        po = pv_qb(pts2)
        normalize_qb(2, *po)

        pts3 = scores_qb(3)
        oproj_qb(2)
        po = pv_qb(pts3)
        normalize_qb(3, *po)
        oproj_qb(3)

    ctx.close()


_NC_CACHE = {}


def build_nc(reps=1):
    if reps in _NC_CACHE:
        return _NC_CACHE[reps]
    nc = bacc.Bacc("TRN2", target_bir_lowering=False, debug=False, num_devices=8)
    t = {}
    for name in ("xq", "xk", "xv"):
        t[name] = nc.dram_tensor(name, [D, S], F32R, kind="ExternalInput")
    for name in ("wq", "wk", "wv"):
        t[name] = nc.dram_tensor(name, [D, DO], F32R, kind="ExternalInput")
    t["wo"] = nc.dram_tensor("wo", [DO, D], F32R, kind="ExternalInput")
    for name in ("bq", "bk"):
        t[name] = nc.dram_tensor(name, [DO, 1], F32, kind="ExternalInput")
    t["bv"] = nc.dram_tensor("bv", [1, DO], F32, kind="ExternalInput")
    t["y"] = nc.dram_tensor("y", [S, D], F32, kind="ExternalOutput")

    with tile.TileContext(nc) as tc:
        emit_kernel(nc, tc, t, reps=reps)
    nc.compile()
    _NC_CACHE[reps] = nc
    return nc


def make_in_maps(q, k, v, Wq, bq, Wk, bk, Wv, bv, Wo, bo):
    in_maps = []
    for c in range(8):
        b = c // 4
        hs = (c % 4) * DO
        in_maps.append(
            {
                "xq": np.ascontiguousarray(q[b].T),
                "xk": np.ascontiguousarray(k[b].T),
                "xv": np.ascontiguousarray(v[b].T),
                "wq": np.ascontiguousarray(Wq[:, hs : hs + DO]),
                "wk": np.ascontiguousarray(Wk[:, hs : hs + DO]),
                "wv": np.ascontiguousarray(Wv[:, hs : hs + DO]),
                "wo": np.ascontiguousarray(Wo[hs : hs + DO, :]),
                "bq": np.ascontiguousarray(bq[hs : hs + DO, None]),
                "bk": np.ascontiguousarray(bk[hs : hs + DO, None]),
                "bv": np.ascontiguousarray(bv[None, hs : hs + DO]),
            }
        )
    return in_maps


def kernel(q, k, v, Wq, bq, Wk, bk, Wv, bv, Wo, bo, _reps=1):
    q = np.asarray(q, dtype=np.float32)
    k = np.asarray(k, dtype=np.float32)
    v = np.asarray(v, dtype=np.float32)
    nc = build_nc(reps=_reps)
    in_maps = make_in_maps(q, k, v, np.asarray(Wq), np.asarray(bq), np.asarray(Wk),
                           np.asarray(bk), np.asarray(Wv), np.asarray(bv),
                           np.asarray(Wo), np.asarray(bo))
    res = run_bass_kernel_spmd(nc, in_maps, list(range(8)))
    B = q.shape[0]
    y = np.zeros((B, S, D), dtype=np.float32)
    for c in range(8):
        y[c // 4] += res.results[c]["y"]
    y += np.asarray(bo, dtype=np.float32)[None, None, :]
    return y


# revision 9
# speedup vs baseline: 1.4111x; 1.2330x over previous
"""Multi-head attention (B=2, S=2048, D=768, H=12) on 8 Trainium2 cores.

Sharding: core c handles batch b=c//4 and heads 3*(c%4)..3*(c%4)+3.
QKV weights column-sharded, Wo row-sharded (Megatron); host sums the 4
partial outputs per batch and adds bo.

v2 schedule (per core):
  - x loaded in [128, seq-chunk] tiles over the sync HWDGE queue, ordered
    xk, xq(b0), xv(b01), xq(b1), xv(b23), xq(b2), xq(b3) so attention
    q-block 0 starts ~25us in while V still streams.
  - QK proj: psum [128,2,512] tile per nb; bank0 = head01 cols (128-wide
    matmuls), bank1 = head2 cols (64-wide); head2 rows duplicated to
    partitions 64:128 for PE row-tile pairing.
  - V proj: moving operand is Wv padded to 256 cols (fp32r needs moving
    >=256 for full PE rate); output evacuated to k-major bf16 V with
    interleaved ones columns (softmax denominator rides the PV matmul).
  - scores: all three heads' S^T matmuls are 64-row PE tiles; head0 at
    partitions 0:63 pairs with head1 at 64:127 in one psum tile (banks
    0/1); head2 pairs its even/odd kpos chunks via duplicated K/Q rows.
    exp (scale=1/8, no max subtraction) on ACT -> bf16 P^T tiles.
  - PV: full-height 128x128-mode matmuls, po[65,512] accumulators per
    head (row 64 = denominator), phased after the q-block's scores so the
    PE stays in one tile mode per phase.
  - normalize: reciprocal on DVE, broadcast via DMA, multiply on Pool;
    head1 shifted to partitions 64:127 by DMA; head2 lands at rows 0:64
    of a zero-padded [128,S] tensor so its 64-row output-proj matmul can
    run in 128x128 mode against a zero-padded Wo[128:192].
  - oproj: py [128,384] (one psum bank), evacuated on Pool, y written
    over the gpsimd SWDGE queue so rep r+1's x loads (sync queue) are
    not stuck behind rep r's y stores.
"""

import sys

sys.path.insert(0, "/opt/trn_rl_repo")

from contextlib import ExitStack

import numpy as np

import concourse.bacc as bacc
import concourse.bass as bass
import concourse.tile as tile
from concourse import mybir
from concourse.bass_utils import run_bass_kernel_spmd

F32 = mybir.dt.float32
F32R = mybir.dt.float32r
BF16 = mybir.dt.bfloat16

S = 2048  # sequence length
D = 768  # model dim
HP = 3  # heads per core
DK = 64  # head dim
DO = HP * DK  # 192 out-cols per core
KT = D // 128  # 6 contraction tiles for projections
NB = S // 512  # 4 sequence blocks of 512
NKT = S // 128  # 16 kpos tiles
G = NKT // 2  # 8 groups of 2 kpos tiles
VW = HP * 65 + 1  # 196: [1|V0|1|V1|1|V2|1] ones interleaved
EXP = mybir.ActivationFunctionType.Exp


def emit_kernel(nc, tc, t, reps=1):
    ctx = ExitStack()
    sb = ctx.enter_context(tc.tile_pool(name="sb", bufs=1))
    xa = ctx.enter_context(tc.tile_pool(name="xa", bufs=1))  # xk then xq
    xb = ctx.enter_context(tc.tile_pool(name="xb", bufs=1))  # xv
    ptp = ctx.enter_context(tc.tile_pool(name="ptp", bufs=1))
    work = ctx.enter_context(tc.tile_pool(name="work", bufs=2))
    ps = ctx.enter_context(tc.tile_pool(name="ps", bufs=1, space=bass.MemorySpace.PSUM))

    # ---- persistent SBUF tensors ----
    wq_sb = sb.tile([128, KT, DO], F32R)
    wk_sb = sb.tile([128, KT, DO], F32R)
    wv_sb = sb.tile([128, KT, 256], F32R)  # cols 192:256 zero (moving>=256)
    wo1_sb = sb.tile([128, D], F32R)  # Wo rows 0:128
    wo2b_sb = sb.tile([128, D], F32R)  # rows 0:64 = Wo[128:192], 64:128 zero
    bq_sb = sb.tile([128, 2], F32)
    bk_sb = sb.tile([128, 2], F32)
    bv_bc = sb.tile([128, DO], F32)  # bv broadcast to 128 partitions
    qt_a = sb.tile([128, S], F32R)  # Q^T rows 0:128 (heads 0,1)
    qt_b = sb.tile([128, S], F32R)  # head 2 at 0:64, dup at 64:128
    kt_a = sb.tile([128, S], F32R)
    kt_b = sb.tile([128, S], F32R)
    v_sb = sb.tile([128, NKT, VW], BF16)  # k-major V with ones cols
    out_a = sb.tile([128, S], F32R)  # normalized attn out^T, heads 0,1
    out_b2 = sb.tile([128, S], F32R)  # head 2 at rows 0:64, rows 64:128 zero

    # ---- one-time loads / inits ----
    nc.sync.dma_start(wq_sb[:], t["wq"].ap().rearrange("(k p) o -> p k o", p=128))
    nc.sync.dma_start(wk_sb[:], t["wk"].ap().rearrange("(k p) o -> p k o", p=128))
    nc.sync.dma_start(
        wv_sb[:, :, 0:DO], t["wv"].ap().rearrange("(k p) o -> p k o", p=128)
    )
    nc.vector.memset(wv_sb[:, :, DO:256].bitcast(F32), 0.0)
    nc.sync.dma_start(wo1_sb[:], t["wo"].ap()[0:128, :])
    nc.sync.dma_start(wo2b_sb[0:64, :], t["wo"].ap()[128:DO, :])
    nc.vector.memset(wo2b_sb[64:128, :].bitcast(F32), 0.0)
    nc.vector.memset(out_b2[64:128, :].bitcast(F32), 0.0)
    nc.sync.dma_start(bq_sb[:, 0:1], t["bq"].ap()[0:128, :])
    nc.sync.dma_start(bq_sb[0:64, 1:2], t["bq"].ap()[128:DO, :])
    nc.sync.dma_start(bq_sb[64:128, 1:2], t["bq"].ap()[128:DO, :])
    nc.sync.dma_start(bk_sb[:, 0:1], t["bk"].ap()[0:128, :])
    nc.sync.dma_start(bk_sb[0:64, 1:2], t["bk"].ap()[128:DO, :])
    nc.sync.dma_start(bk_sb[64:128, 1:2], t["bk"].ap()[128:DO, :])
    nc.sync.dma_start(bv_bc[:], t["bv"].ap().unsqueeze(1).to_broadcast([1, 128, DO]))
    for oc in (0, 65, 130, 195):
        nc.vector.memset(v_sb[:, :, oc : oc + 1], 1.0)

    def dma_x_nb(xdram, pool, pfx, nb):
        """Load x[:, nb*512:(nb+1)*512] as 6 [128,512] chunk tiles."""
        xts = []
        for k in range(KT):
            xt = pool.tile(
                [128, 512], F32R, name=f"{pfx}{k}", tag=f"{pfx}{k}", bufs=2
            )
            nc.sync.dma_start(
                xt[:], xdram.ap()[k * 128 : k * 128 + 128, nb * 512 : nb * 512 + 512]
            )
            xts.append(xt)
        return xts

    def proj_qk_nb(xts, w_sb, b_sb, dst_a, dst_b, nb):
        nb0 = nb * 512
        pq = ps.tile([128, 2, 512], F32, tag="ss", bufs=2, name="pq")
        for k in range(KT):
            nc.tensor.matmul(
                pq[:, 0, :], w_sb[:, k, 0:128], xts[k][:],
                start=(k == 0), stop=(k == KT - 1),
            )
        nc.vector.tensor_scalar_add(
            dst_a[:, nb0 : nb0 + 512], pq[:, 0, :], b_sb[:, 0:1]
        )
        for k in range(KT):
            nc.tensor.matmul(
                pq[0:64, 1, :], w_sb[:, k, 128:DO], xts[k][:],
                start=(k == 0), stop=(k == KT - 1),
            )
        nc.vector.tensor_scalar_add(
            dst_b[0:64, nb0 : nb0 + 512], pq[0:64, 1, :], b_sb[0:64, 1:2]
        )
        nc.sync.dma_start(
            dst_b[64:128, nb0 : nb0 + 512], dst_b[0:64, nb0 : nb0 + 512]
        )

    def proj_v_nb(xts, nb):
        # tag "py" (not "ss") so the V pipeline never waits on score-tile
        # exp frees; bufs=1 serializes st on the previous evac, which hides
        # under the exp stream anyway.
        for sl in range(4):
            st = nb * 4 + sl
            pv = ps.tile([128, 256], F32, tag="py", bufs=1, name="pv")
            for k in range(KT):
                nc.tensor.matmul(
                    pv[:], xts[k][:, sl * 128 : sl * 128 + 128],
                    wv_sb[:, k, :], start=(k == 0), stop=(k == KT - 1),
                )
            dst = v_sb[:, st, 1:196].rearrange("p (h c) -> p h c", h=HP)[:, :, 0:64]
            nc.vector.tensor_add(
                dst,
                pv[:, 0:DO].rearrange("p (h c) -> p h c", h=HP),
                bv_bc[:].rearrange("p (h c) -> p h c", h=HP),
            )

    def vslice(kt_i, h):
        return v_sb[:, kt_i, 1 + h * 65 : 1 + h * 65 + 65]

    def scores_qb(qb):
        """All 3 heads' S^T for q-block qb; exp into bf16 P^T tiles.

        Returns pts[g] = (ptx, pty, ptz): ptx[:,j,:] = P^T of head j for
        kpos chunk 2g; pty for chunk 2g+1; ptz[:,0,:]/[:,1,:] = head 2
        chunks 2g / 2g+1."""
        q0 = qb * 512
        pts = []
        for g in range(G):
            ke, ko = 2 * g * 128, (2 * g + 1) * 128
            ssx = ps.tile([128, 2, 512], F32, tag="ss", bufs=2, name="ssx")
            nc.tensor.matmul(
                ssx[:, 0, :], kt_a[0:64, ke : ke + 128], qt_a[0:64, q0 : q0 + 512],
                start=True, stop=True,
            )
            nc.tensor.matmul(
                ssx[:, 1, :], kt_a[64:128, ke : ke + 128], qt_a[64:128, q0 : q0 + 512],
                start=True, stop=True,
            )
            ptx = ptp.tile([128, 2, 512], BF16, name="ptx", tag="pt", bufs=12)
            nc.scalar.activation(ptx[:], ssx[:], EXP, scale=0.125)
            ssy = ps.tile([128, 2, 512], F32, tag="ss", bufs=2, name="ssy")
            nc.tensor.matmul(
                ssy[:, 0, :], kt_a[0:64, ko : ko + 128], qt_a[0:64, q0 : q0 + 512],
                start=True, stop=True,
            )
            nc.tensor.matmul(
                ssy[:, 1, :], kt_a[64:128, ko : ko + 128], qt_a[64:128, q0 : q0 + 512],
                start=True, stop=True,
            )
            pty = ptp.tile([128, 2, 512], BF16, name="pty", tag="pt", bufs=12)
            nc.scalar.activation(pty[:], ssy[:], EXP, scale=0.125)
            ssz = ps.tile([128, 2, 512], F32, tag="ss", bufs=2, name="ssz")
            nc.tensor.matmul(
                ssz[:, 0, :], kt_b[0:64, ke : ke + 128], qt_b[0:64, q0 : q0 + 512],
                start=True, stop=True,
            )
            nc.tensor.matmul(
                ssz[:, 1, :], kt_b[64:128, ko : ko + 128], qt_b[64:128, q0 : q0 + 512],
                start=True, stop=True,
            )
            ptz = ptp.tile([128, 2, 512], BF16, name="ptz", tag="pt", bufs=12)
            nc.scalar.activation(ptz[:], ssz[:], EXP, scale=0.125)
            pts.append((ptx, pty, ptz))
        return pts

    def pv_qb(pts):
        """P^T @ [V|1] accumulated over kpos; returns po per head."""
        po0 = ps.tile([65, 512], F32, tag="po0", bufs=1, name="po0")
        po1 = ps.tile([65, 512], F32, tag="po1", bufs=1, name="po1")
        po2 = ps.tile([65, 512], F32, tag="po2", bufs=1, name="po2")
        for g in range(G):
            ptx, pty, ptz = pts[g]
            ke, ko = 2 * g, 2 * g + 1
            st, sp = g == 0, g == G - 1
            nc.tensor.matmul(po0[:], vslice(ke, 0), ptx[:, 0, :],
                             start=st, stop=False, skip_group_check=True)
            nc.tensor.matmul(po0[:], vslice(ko, 0), pty[:, 0, :],
                             start=False, stop=sp, skip_group_check=True)
            nc.tensor.matmul(po1[:], vslice(ke, 1), ptx[:, 1, :],
                             start=st, stop=False, skip_group_check=True)
            nc.tensor.matmul(po1[:], vslice(ko, 1), pty[:, 1, :],
                             start=False, stop=sp, skip_group_check=True)
            nc.tensor.matmul(po2[:], vslice(ke, 2), ptz[:, 0, :],
                             start=st, stop=False, skip_group_check=True)
            nc.tensor.matmul(po2[:], vslice(ko, 2), ptz[:, 1, :],
                             start=False, stop=sp, skip_group_check=True)
        return po0, po1, po2

    def normalize_qb(qb, po0, po1, po2):
        q0 = qb * 512
        for h, po in ((0, po0), (1, po1), (2, po2)):
            dtmp = work.tile([65, 512], F32, name="dtmp", tag="dtmp")
            nc.vector.reciprocal(dtmp[64:65, :], po[64:65, :])
            dbc = work.tile([64, 512], F32, name="dbc", tag="dbc")
            nc.sync.dma_start(
                dbc[:], dtmp[64:65, :].unsqueeze(1).to_broadcast([1, 64, 512])
            )
            if h == 0:
                nc.vector.tensor_mul(out_a[0:64, q0 : q0 + 512], po[0:64, :], dbc[:])
            elif h == 2:
                nc.vector.tensor_mul(out_b2[0:64, q0 : q0 + 512], po[0:64, :], dbc[:])
            else:
                nsb = work.tile([64, 512], F32R, name="nsb", tag="nsb")
                nc.vector.tensor_mul(nsb[:], po[0:64, :], dbc[:])
                nc.sync.dma_start(out_a[64:128, q0 : q0 + 512], nsb[:])

    def oproj_qb(qb):
        q0 = qb * 512
        for ss in range(4):
            s0 = q0 + ss * 128
            ysb = work.tile([128, D], F32, name="ysb", tag="ysb")
            for half in range(2):
                c0 = half * 384
                py = ps.tile([128, 384], F32, tag="py", bufs=1, name="py")
                nc.tensor.matmul(
                    py[:], out_a[:, s0 : s0 + 128], wo1_sb[:, c0 : c0 + 384],
                    start=True, stop=False,
                )
                nc.tensor.matmul(
                    py[:], out_b2[:, s0 : s0 + 128], wo2b_sb[:, c0 : c0 + 384],
                    start=False, stop=True,
                )
                nc.vector.tensor_copy(ysb[:, c0 : c0 + 384], py[:])
            nc.gpsimd.dma_start(t["y"].ap()[s0 : s0 + 128, :], ysb[:])

    def load_proj_kq0():
        """x(q-block0) + all of K: the lead-in for a rep's attention."""
        xts = dma_x_nb(t["xq"], xa, "a", 0)
        proj_qk_nb(xts, wq_sb, bq_sb, qt_a, qt_b, 0)
        for nb in range(NB):
            xts = dma_x_nb(t["xk"], xa, "a", nb)
            proj_qk_nb(xts, wk_sb, bk_sb, kt_a, kt_b, nb)

    load_proj_kq0()
    for rep in range(reps):
        pts0 = scores_qb(0)

        for nb in range(2):
            xts = dma_x_nb(t["xv"], xb, "v", nb)
            proj_v_nb(xts, nb)
        xq1 = dma_x_nb(t["xq"], xa, "a", 1)
        xq2 = dma_x_nb(t["xq"], xa, "a", 2)
        for nb in range(2, NB):
            xts = dma_x_nb(t["xv"], xb, "v", nb)
            proj_v_nb(xts, nb)
        proj_qk_nb(xq1, wq_sb, bq_sb, qt_a, qt_b, 1)
        proj_qk_nb(xq2, wq_sb, bq_sb, qt_a, qt_b, 2)

        po = pv_qb(pts0)
        normalize_qb(0, *po)

        pts1 = scores_qb(1)
        oproj_qb(0)
        po = pv_qb(pts1)
        normalize_qb(1, *po)

        pts2 = scores_qb(2)
        oproj_qb(1)
        xts = dma_x_nb(t["xq"], xa, "a", 3)
        proj_qk_nb(xts, wq_sb, bq_sb, qt_a, qt_b, 3)
        po = pv_qb(pts2)
        normalize_qb(2, *po)

        pts3 = scores_qb(3)
        oproj_qb(2)
        if rep < reps - 1:
            # next rep's lead-in runs in the exp-qb3 window
            load_proj_kq0()
        po = pv_qb(pts3)
        normalize_qb(3, *po)
        oproj_qb(3)

    ctx.close()


_NC_CACHE = {}


def build_nc(reps=1):
    if reps in _NC_CACHE:
        return _NC_CACHE[reps]
    nc = bacc.Bacc("TRN2", target_bir_lowering=False, debug=False, num_devices=8)
    t = {}
    for name in ("xq", "xk", "xv"):
        t[name] = nc.dram_tensor(name, [D, S], F32R, kind="ExternalInput")
    for name in ("wq", "wk", "wv"):
        t[name] = nc.dram_tensor(name, [D, DO], F32R, kind="ExternalInput")
    t["wo"] = nc.dram_tensor("wo", [DO, D], F32R, kind="ExternalInput")
    for name in ("bq", "bk"):
        t[name] = nc.dram_tensor(name, [DO, 1], F32, kind="ExternalInput")
    t["bv"] = nc.dram_tensor("bv", [1, DO], F32, kind="ExternalInput")
    t["y"] = nc.dram_tensor("y", [S, D], F32, kind="ExternalOutput")

    with tile.TileContext(nc) as tc:
        emit_kernel(nc, tc, t, reps=reps)
    nc.compile()
    _NC_CACHE[reps] = nc
    return nc


def make_in_maps(q, k, v, Wq, bq, Wk, bk, Wv, bv, Wo, bo):
    in_maps = []
    for c in range(8):
        b = c // 4
        hs = (c % 4) * DO
        in_maps.append(
            {
                "xq": np.ascontiguousarray(q[b].T),
                "xk": np.ascontiguousarray(k[b].T),
                "xv": np.ascontiguousarray(v[b].T),
                "wq": np.ascontiguousarray(Wq[:, hs : hs + DO]),
                "wk": np.ascontiguousarray(Wk[:, hs : hs + DO]),
                "wv": np.ascontiguousarray(Wv[:, hs : hs + DO]),
                "wo": np.ascontiguousarray(Wo[hs : hs + DO, :]),
                "bq": np.ascontiguousarray(bq[hs : hs + DO, None]),
                "bk": np.ascontiguousarray(bk[hs : hs + DO, None]),
                "bv": np.ascontiguousarray(bv[None, hs : hs + DO]),
            }
        )
    return in_maps


def kernel(q, k, v, Wq, bq, Wk, bk, Wv, bv, Wo, bo, _reps=1):
    q = np.asarray(q, dtype=np.float32)
    k = np.asarray(k, dtype=np.float32)
    v = np.asarray(v, dtype=np.float32)
    nc = build_nc(reps=_reps)
    in_maps = make_in_maps(q, k, v, np.asarray(Wq), np.asarray(bq), np.asarray(Wk),
                           np.asarray(bk), np.asarray(Wv), np.asarray(bv),
                           np.asarray(Wo), np.asarray(bo))
    res = run_bass_kernel_spmd(nc, in_maps, list(range(8)))
    B = q.shape[0]
    y = np.zeros((B, S, D), dtype=np.float32)
    for c in range(8):
        y[c // 4] += res.results[c]["y"]
    y += np.asarray(bo, dtype=np.float32)[None, None, :]
    return y
